# revision 33
# baseline (speedup 1.0000x reference)
"""Trainium2 Bass kernel for nn_Deepset GNN message-passing problem.

Computation:
    h  = relu(x @ W1 + b1)          # [N, 64]   (x: [400000, 1024])
    h2 = h @ W2 + b2                # [N, 64]
    pooled = segment_mean(h2, batch, 512)
    z = (pooled @ W3 + b3) @ W4 + b4
    out = softmax(z, axis=0)        # [512, 2]

Device does the dominant work: h = relu(x@W1+b1) and the per-graph
segment-sum of h. Everything downstream of the [512, 64] segment sums
(~2 MFLOP) runs on host (W2 commutes with the mean pool).

Sharding: data-parallel over nodes, 50000 nodes/core on 8 cores.

Device pipeline per core (fp8 DoubleRow compute, fp32 accumulation):
  - x shard is cast to fp8e4m3 and packed tile-major on host so each
    512-node tile is one fully contiguous [128 partitions x 4KB] DMA.
  - W1 is pre-scaled by 64 into fp8e4m3 range; the relu activation
    un-scales with scale=1/64 (out = relu(psum/64 + b1)).
  - PE: per 512-node tile, hT[64,512] = 4 DoubleRow matmuls, each
    contracting 256 features (lhsT [128,2,64], rhs [128,2,512]).
  - ScalarE: relu+bias (PSUM->SBUF bf16) with accum_out giving the
    per-tile row-sum T_t[64] for free.
  - Segment-sum exploits sorted `batch` + min-graph-size > tile size:
    each 512-node tile spans at most 2 consecutive graphs. DVE runs a
    prefix-sum (tensor_tensor_scan) of relu(h) along the node axis;
    GPSIMD indirect_copy gathers the cumsum column at the (data-driven)
    graph-boundary index s_t-1 => B_t[64]. Host combines:
    S[g_left] += B_t, S[g_right] += T_t - B_t.  No PE transposes, no
    one-hot matmuls: the tensor engine runs only the main matmuls.
"""

import numpy as np

N_NODES = 400000
D_FEAT = 1024
HIDDEN = 64
NUM_GRAPHS = 512
N_CORES = 8
NPC = N_NODES // N_CORES        # 50000 nodes per core
TILE_N = 512                    # nodes per PE tile
N_PAD = 50176                   # 98 * 512
N_TILES = N_PAD // TILE_N       # 98
KC = D_FEAT // 128              # 8 contraction chunks
W1_SCALE = 64.0                 # W1 pre-scale into fp8e4m3 range

LAST_RESULT = None              # BassKernelResults of the last run (for profiling)


def _build_nc(d_feat=D_FEAT, n_pad=N_PAD, tile_n=TILE_N, hidden=HIDDEN,
              repeat=1, xp_bufs=16, dma_split=1, mode="full",
              dma_engines=("sync",)):
    import concourse.bass as bass
    import concourse.bacc as bacc
    import concourse.tile as tile
    from concourse import library_config, mybir
    from contextlib import ExitStack

    dt = mybir.dt
    kc = d_feat // 128
    n_tiles = n_pad // tile_n

    nc = bacc.Bacc("TRN2", target_bir_lowering=False, debug=False)
    xT = nc.declare_dram_parameter("xT", [n_tiles, 128, kc * tile_n],
                                   dt.float8e4, isOutput=False)
    w1 = nc.declare_dram_parameter("w1", [d_feat, hidden], dt.float8e4,
                                   isOutput=False)
    b1 = nc.declare_dram_parameter("b1", [hidden, 1], dt.float32, isOutput=False)
    # per tile: col 2t = gather index (s_t - 1 at partition 16k, 511 filler
    # elsewhere), col 2t+1 = filler so each slice stays 4-byte aligned
    bidx = nc.declare_dram_parameter("bidx", [128, 2 * n_tiles], dt.int16,
                                     isOutput=False)
    # per tile, 16 gather columns: col 16t = boundary cumsum column B_t,
    # col 16t+1 = total T_t, rest filler (contiguous DMA beats compaction)
    sout = nc.declare_dram_parameter("sout", [hidden, 16 * n_tiles], dt.float32,
                                     isOutput=True)

    w1_r = w1[:, :].rearrange("(c p) h -> p c h", p=128)

    with ExitStack() as ctx:
        tc = ctx.enter_context(tile.TileContext(nc))
        const = ctx.enter_context(tc.tile_pool(name="const", bufs=1))
        xp = ctx.enter_context(tc.tile_pool(name="xp", bufs=xp_bufs))
        htp = ctx.enter_context(tc.tile_pool(name="htp", bufs=4,
                                             space=bass.MemorySpace.PSUM))
        hts = ctx.enter_context(tc.tile_pool(name="hts", bufs=3))

        nc.gpsimd.load_library(library_config.ap_gather)

        w1_sb = const.tile([128, kc, hidden], dt.float8e4)
        nc.sync.dma_start(w1_sb[:], w1_r)
        b1_sb = const.tile([hidden, 1], dt.float32)
        nc.sync.dma_start(b1_sb[:], b1[:, :])
        bidx_sb = const.tile([128, 2 * n_tiles], dt.int16)
        nc.sync.dma_start(bidx_sb[:], bidx[:, :])

        # Gather output: 16 columns per tile (0 = B_t, 1 = T_t, rest junk).
        NGI = 16
        b_sb = const.tile([128, NGI * n_tiles], dt.float32)
        # Scan buffers (manually rotated). 128 partitions because ap_gather
        # works on 128-partition data; the scan writes the lower `hidden`
        # rows, the memset keeps the junk rows finite.
        sc_tiles = [const.tile([128, tile_n], dt.float32, name=f"sc{i}")
                    for i in range(3)]
        for s in sc_tiles:
            nc.vector.memset(s[:], 0.0)

        engs = [getattr(nc, e) for e in dma_engines]
        ndma = 0
        xt0 = None
        assert n_tiles % 2 == 0
        for r in range(repeat):  # repeat>1 is a bench-only mode
            for tp in range(n_tiles // 2):
                pair = (2 * tp, 2 * tp + 1)
                if mode == "peonly":
                    if xt0 is None:
                        xt0 = const.tile([128, 2, kc, tile_n], dt.float8e4)
                        xsrc = xT[0:2, :, :].rearrange("t p (c n) -> p t c n",
                                                       c=kc)
                        nc.sync.dma_start(xt0[:], xsrc)
                    x2 = xt0
                else:
                    # one 1MB DMA covers both tiles of the pair
                    x2 = xp.tile([128, 2, kc, tile_n], dt.float8e4)
                    xsrc = xT[pair[0]:pair[0] + 2, :, :].rearrange(
                        "t p (c n) -> p t c n", c=kc)
                    ks = 2 // dma_split if dma_split <= 2 else 1
                    for s in range(dma_split if dma_split <= 2 else 2):
                        engs[ndma % len(engs)].dma_start(
                            x2[:, s * ks:(s + 1) * ks, :, :],
                            xsrc[:, s * ks:(s + 1) * ks, :, :])
                        ndma += 1
                if mode == "dmaonly":
                    continue

                # weight-grouped emission: both tiles' matmuls for one k-pair
                # are adjacent, so the stationary operand only changes every
                # second matmul.
                pss = []
                for j in range(2):
                    ht_ps = htp.tile([hidden, tile_n], dt.float32,
                                     name=f"ht_ps{j}")
                    pss.append(ht_ps)
                for k in range(kc // 2):
                    for j in range(2):
                        nc.tensor.matmul(
                            pss[j][:], w1_sb[:, 2 * k:2 * k + 2, :],
                            x2[:, j, 2 * k:2 * k + 2, :],
                            start=(k == 0), stop=(k == kc // 2 - 1),
                            perf_mode=mybir.MatmulPerfMode.DoubleRow)

                for j, t in enumerate(pair):
                    ht_sb = hts.tile([hidden, tile_n], dt.bfloat16)
                    nc.scalar.activation(ht_sb[:], pss[j][:],
                                         mybir.ActivationFunctionType.Relu,
                                         bias=b1_sb[:], scale=1.0 / W1_SCALE)

                    sc = sc_tiles[t % 3]
                    nc.vector.tensor_tensor_scan(sc[:hidden, :], ht_sb[:],
                                                 ht_sb[:], 0.0,
                                                 mybir.AluOpType.add,
                                                 mybir.AluOpType.bypass)

                    nc.gpsimd.ap_gather(b_sb[:, NGI * t:NGI * (t + 1)], sc[:],
                                        bidx_sb[:, 2 * t:2 * t + 1],
                                        channels=128, num_elems=tile_n, d=1,
                                        num_idxs=NGI)

        if mode == "dmaonly":
            nc.any.memset(b_sb[:hidden, :], 0.0)
        nc.sync.dma_start(sout[:, :], b_sb[:hidden, :])

    nc.compile()
    return nc


def _f8dt():
    import ml_dtypes
    return np.dtype(ml_dtypes.float8_e4m3)


def _prep_w(W1, b1):
    """Weight-side input-map entries (shared by kernel() and test bench)."""
    w1_np = (np.asarray(W1, np.float32) * W1_SCALE).astype(_f8dt())
    b1_np = np.asarray(b1, np.float32).reshape(HIDDEN, 1).copy()
    return {"w1": w1_np, "b1": b1_np}


def _core_tiles(b):
    """Per-tile (g_left, split s_t, g_right, n_real) for one core's sorted
    batch slice b [NPC]."""
    out = []
    for t in range(N_TILES):
        lo = t * TILE_N
        nt = min(TILE_N, NPC - lo)
        seg = b[lo:lo + nt]
        gl = int(seg[0])
        s = int(np.searchsorted(seg, gl, side="right"))
        gr = int(seg[s]) if s < nt else -1
        if s < nt:
            # at most one boundary per tile (min graph size > TILE_N)
            assert int(seg[-1]) == gr, (
                f"tile {t}: >2 graphs in one tile ({gl}, {gr}, {int(seg[-1])})")
        out.append((gl, s, gr, nt))
    return out


def _prep_inputs(x, batch):
    """Per-core input maps + per-core tile split info for the host combine."""
    f8 = _f8dt()
    batch = np.asarray(batch, dtype=np.int64)

    in_maps = []
    tile_infos = []
    for i in range(N_CORES):
        lo, hi = i * NPC, (i + 1) * NPC
        xs = np.zeros((N_PAD, D_FEAT), dtype=f8)
        xs[:NPC] = x[lo:hi].astype(f8)
        # tile-major pack: xTt[t, p, c*TILE_N + n] = x[t*TILE_N + n, c*128 + p]
        # so each 512-node tile is one fully-contiguous [128, 4KB] DMA.
        xT = np.ascontiguousarray(
            xs.reshape(N_TILES, TILE_N, KC, 128).transpose(0, 3, 2, 1)
        ).reshape(N_TILES, 128, KC * TILE_N)

        info = _core_tiles(batch[lo:hi])
        tile_infos.append(info)
        # gather index layout: partition 16k slot 0 -> s_t-1 (B column),
        # partition 16k+1 slot 0 -> TILE_N-1 (T column), filler elsewhere.
        bidx = np.full((128, 2 * N_TILES), TILE_N - 1, np.int16)
        for t, (gl, s, gr, nt) in enumerate(info):
            bidx[0::16, 2 * t] = s - 1

        in_maps.append({"xT": xT, "bidx": bidx})
    return in_maps, tile_infos


def kernel(x, batch, W1, b1, W2, b2, W3, b3, W4, b4):
    global LAST_RESULT
    from concourse.bass_utils import run_bass_kernel_spmd

    x = np.asarray(x)
    batch = np.asarray(batch)

    in_maps, tile_infos = _prep_inputs(x, batch)
    w_map = _prep_w(W1, b1)
    for m in in_maps:
        m.update(w_map)

    nc = _build_nc()
    res = run_bass_kernel_spmd(nc, in_maps, list(range(N_CORES)))
    LAST_RESULT = res

    # Host-side: combine per-tile partial sums, then the tiny head.
    relu_b1 = np.maximum(np.asarray(b1, np.float64), 0.0)
    S = np.zeros((NUM_GRAPHS, HIDDEN), np.float64)
    for i in range(N_CORES):
        out = np.asarray(res.results[i]["sout"], np.float64)  # [64, 16*N_TILES]
        B = out[:, 0::16]
        T = out[:, 1::16]
        for t, (gl, s, gr, nt) in enumerate(tile_infos[i]):
            S[gl] += B[:, t]
            rest = T[:, t] - B[:, t]
            if nt < TILE_N:
                rest -= (TILE_N - nt) * relu_b1  # zero-padded tail nodes
            if gr >= 0:
                S[gr] += rest

    cnt = np.bincount(batch.astype(np.int64), minlength=NUM_GRAPHS).astype(np.float64)
    meanh = S / np.maximum(cnt, 1.0)[:, None]
    pooled = meanh @ np.asarray(W2, np.float64) + np.asarray(b2, np.float64)
    pooled *= (cnt > 0)[:, None]  # empty graphs pool to exactly zero in the reference
    z = pooled @ np.asarray(W3, np.float64) + np.asarray(b3, np.float64)
    z = z @ np.asarray(W4, np.float64) + np.asarray(b4, np.float64)
    z -= z.max(axis=0, keepdims=True)
    e = np.exp(z)
    out = e / e.sum(axis=0, keepdims=True)
    return out.astype(np.float32)
